# revision 15
# baseline (speedup 1.0000x reference)
"""GSA block on 8 NeuronCores via Bass/Tile.

Sharding: core c -> batch b=c//2, head-pair hp=c%2 (heads 2hp, 2hp+1).
Device does projections (float32r), chunkwise recurrence (C=128, 16 chunks,
mid-centered gate cumsum), RMSNorm and o@Wo partial products.
Host pre-transposes x, slices weights, sums core-pair partial outputs.
"""
import os
import sys
import numpy as np

sys.path.insert(0, '/opt/trn_rl_repo')

B, T, D = 4, 2048, 1024
H, K, V, M = 4, 256, 256, 64
HP = 2                     # heads per core
KP = HP * K                # 512 projection cols per core (q/k/v)
MP = HP * M                # 128 gate cols per core
GATE_NORM = 8.0
EPS = 1e-5
SCALE = K ** -0.5
C = 128                    # chunk length
NCH = T // C               # 16 chunks
NKT = D // 128             # 8 contraction tiles over D
DT_T = 512                 # free-dim tile for projections

last_exec_time_ns = None
_cache = {}


def _build_consts():
    j = np.arange(C)[:, None]
    c = np.arange(C)[None, :]
    mask_ut = (c >= j).astype(np.float32)          # [j, c] keep j<=c
    lt2 = mask_ut - (j < C // 2)                   # centered cumsum matrix
    eye = np.eye(C, dtype=np.float32)
    half_col = (j < C // 2).astype(np.float32)     # [C, 1]
    return np.concatenate([mask_ut, lt2, eye, half_col.reshape(C, 1)], axis=1)


def _build_program():
    import concourse.bacc as bacc
    import concourse.tile as tile
    import concourse.mybir as mybir

    f32, f32r, bf16 = mybir.dt.float32, mybir.dt.float32r, mybir.dt.bfloat16
    AF = mybir.ActivationFunctionType
    ALU = mybir.AluOpType
    AX = mybir.AxisListType

    nc = bacc.Bacc("TRN2", target_bir_lowering=False, debug=False,
                   num_devices=8)

    xT_d = nc.dram_tensor("xT", [D, T], f32r, kind="ExternalInput").ap()
    wq_d = nc.dram_tensor("wq", [D, KP], f32r, kind="ExternalInput").ap()
    wk_d = nc.dram_tensor("wk", [D, KP], f32r, kind="ExternalInput").ap()
    wv_d = nc.dram_tensor("wv", [D, KP], f32r, kind="ExternalInput").ap()
    wf_d = nc.dram_tensor("wf", [D, MP], f32r, kind="ExternalInput").ap()
    wo_d = nc.dram_tensor("wo", [KP, D], bf16, kind="ExternalInput").ap()
    cn_d = nc.dram_tensor("cn", [C, 3 * C + 1], f32, kind="ExternalInput").ap()
    y_d = nc.dram_tensor("y", [T, D], f32, kind="ExternalOutput").ap()

    with tile.TileContext(nc) as tc:
        with tc.tile_pool(name="persist", bufs=1) as pp:
            cn = pp.tile([C, 3 * C + 1], f32)
            nc.sync.dma_start(cn[:], cn_d[:])
            mask_ut = cn[:, 0:C]
            lt2 = cn[:, C:2 * C]
            eye = cn[:, 2 * C:3 * C]
            half_col = cn[:, 3 * C:3 * C + 1]
            ones1 = cn[0:1, 0:C]              # row 0 of mask_ut = ones
            eye_bf = pp.tile([C, C], bf16)
            nc.any.tensor_copy(eye_bf[:], cn[:, 2 * C:3 * C])
            epsc = pp.tile([C, 1], f32)
            nc.vector.memset(epsc[:], EPS)
            onec = pp.tile([C, 1], f32)
            nc.vector.memset(onec[:], 1.0)

            # ---- persistent activation storage ----
            qT = pp.tile([128, 4 * T], bf16)   # 4 col-tiles of [128, T]
            kT = pp.tile([128, 4 * T], bf16)
            vn = pp.tile([128, NCH * KP], bf16)  # 16 t-tiles of [128, 512]
            fT = pp.tile([128, T], f32)
            oT = pp.tile([128, 4 * T], bf16)
            wo = pp.tile([128, 4 * D], bf16)   # 4 kt tiles of [128, 1024]
            for kt in range(4):
                nc.sync.dma_start(wo[:, kt * D:(kt + 1) * D],
                                  wo_d[kt * 128:(kt + 1) * 128, :])

            with (
                tc.tile_pool(name="proj", bufs=1) as jp,
                tc.tile_pool(name="proj_ps", bufs=8, space="PSUM") as jps,
            ):
                xT = jp.tile([128, NKT * T], f32r)     # 8 kt tiles of [128, T]
                wq = jp.tile([128, NKT * KP], f32r)
                wk = jp.tile([128, NKT * KP], f32r)
                wv = jp.tile([128, NKT * KP], f32r)
                wf = jp.tile([128, NKT * MP], f32r)
                for kt in range(NKT):
                    rs = slice(kt * 128, (kt + 1) * 128)
                    nc.sync.dma_start(xT[:, kt * T:(kt + 1) * T], xT_d[rs, :])
                    nc.sync.dma_start(wq[:, kt * KP:(kt + 1) * KP],
                                      wq_d[rs, :])
                    nc.sync.dma_start(wk[:, kt * KP:(kt + 1) * KP],
                                      wk_d[rs, :])
                    nc.sync.dma_start(wv[:, kt * KP:(kt + 1) * KP],
                                      wv_d[rs, :])
                    nc.sync.dma_start(wf[:, kt * MP:(kt + 1) * MP],
                                      wf_d[rs, :])

                def xTb(kt, t0, ts):
                    return xT[:, kt * T + t0:kt * T + t0 + ts]

                # qT/kT: [KP, T] transposed outputs; 4 col-tiles x 4 t-tiles
                for w_sb, outT, act in ((wq, qT, AF.Silu), (wk, kT, AF.Silu)):
                    for cc in range(4):           # out partition tile (q cols)
                        for tt in range(4):       # free t tile
                            ps = jps.tile([128, DT_T], f32, tag="pjps")
                            for kt in range(NKT):
                                lhs = w_sb[:, kt * KP + cc * 128:
                                           kt * KP + (cc + 1) * 128]
                                nc.tensor.matmul(
                                    ps[:], lhs,
                                    xTb(kt, tt * DT_T, DT_T),
                                    start=(kt == 0), stop=(kt == NKT - 1))
                            nc.scalar.activation(
                                outT[:, cc * T + tt * DT_T:
                                     cc * T + (tt + 1) * DT_T], ps[:], act)
                # v natural: [T, KP]; 16 t-tiles [128, 512]
                for tt in range(NCH):
                    ps = jps.tile([128, KP], f32, tag="pjps")
                    for kt in range(NKT):
                        lhs = xT[:, kt * T + tt * 128:kt * T + (tt + 1) * 128]
                        nc.tensor.matmul(
                            ps[:], lhs,
                            wv[:, kt * KP:(kt + 1) * KP],
                            start=(kt == 0), stop=(kt == NKT - 1))
                    nc.any.tensor_copy(vn[:, tt * KP:(tt + 1) * KP], ps[:])
                # fT: [MP, T] transposed; raw -> logsigmoid/GATE_NORM
                for tt in range(4):
                    ps = jps.tile([128, DT_T], f32, tag="pjps")
                    for kt in range(NKT):
                        lhs = wf[:, kt * MP:(kt + 1) * MP]
                        nc.tensor.matmul(
                            ps[:], lhs,
                            xTb(kt, tt * DT_T, DT_T),
                            start=(kt == 0), stop=(kt == NKT - 1))
                    sp = jp.tile([128, DT_T], f32, tag="fsp")
                    nc.scalar.activation(sp[:], ps[:], AF.Exp, scale=-1.0)
                    sp2 = jp.tile([128, DT_T], f32, tag="fsp2")
                    nc.scalar.activation(sp2[:], sp[:], AF.Ln, bias=onec[:])
                    nc.vector.tensor_scalar_mul(
                        fT[:, tt * DT_T:(tt + 1) * DT_T], sp2[:],
                        -1.0 / GATE_NORM)

            # ---- recurrence ----
            with (
                tc.tile_pool(name="st", bufs=2) as stp,
                tc.tile_pool(name="ck", bufs=3) as ckp,
                tc.tile_pool(name="ck1", bufs=3) as ck1,
                tc.tile_pool(name="ps_g", bufs=1, space="PSUM") as psg,
                tc.tile_pool(name="ps_tp", bufs=2, space="PSUM") as pst,
                tc.tile_pool(name="ps_gr", bufs=2, space="PSUM") as psgr,
                tc.tile_pool(name="ps_lg", bufs=1, space="PSUM") as pslg,
                tc.tile_pool(name="ps_big", bufs=2, space="PSUM") as psbig,
            ):
                state0 = stp.tile([128, 512], f32, tag="state")
                nc.vector.memset(state0[:], 0.0)
                state = state0
                for i in range(NCH):
                    t0 = i * C
                    # gates
                    fnP = psg.tile([C, 128], f32, tag="g")
                    nc.tensor.transpose(fnP[:], fT[:, t0:t0 + C], eye)
                    fn = ckp.tile([C, 128], f32, tag="fn")
                    nc.any.tensor_copy(fn[:], fnP[:])
                    cumP = psg.tile([C, 128], f32, tag="g")
                    nc.tensor.matmul(cumP[:], lt2, fn[:])
                    midP = psg.tile([1, 128], f32, tag="g")
                    nc.tensor.matmul(midP[:], half_col, fn[:])
                    ai = ckp.tile([C, 128], f32, tag="ai")
                    nc.scalar.activation(ai[:], cumP[:], AF.Exp)
                    ain = ckp.tile([C, 128], f32, tag="ain")
                    nc.scalar.activation(ain[:], cumP[:], AF.Exp, scale=-1.0)
                    emid = ckp.tile([1, 128], f32, tag="emid")
                    nc.scalar.activation(emid[:], midP[:], AF.Exp)
                    ef = ckp.tile([C, 128], f32, tag="ef")
                    nc.scalar.activation(ef[:], fn[:], AF.Exp)
                    sn = ckp.tile([C, 128], f32, tag="sn")
                    nc.vector.tensor_scalar(sn[:], ef[:], -1.0, 1.0,
                                            op0=ALU.mult, op1=ALU.add)
                    stil = ckp.tile([C, 128], bf16, tag="stil")
                    nc.vector.tensor_mul(stil[:], sn[:], ain[:])
                    alP = psg.tile([1, 128], f32, tag="g")
                    nc.tensor.matmul(alP[:], lt2[:, C - 1:C], fn[:])
                    ail = ckp.tile([1, 128], f32, tag="ail")
                    nc.scalar.activation(ail[:], alP[:], AF.Exp)
                    atot = ckp.tile([1, 128], f32, tag="atot")
                    nc.vector.tensor_mul(atot[:], ail[:], emid[:])
                    bcP = psg.tile([C, 128], f32, tag="g")
                    nc.tensor.matmul(bcP[:], ones1, ail[:])
                    sa = ckp.tile([C, 128], bf16, tag="sa")
                    nc.vector.tensor_mul(sa[:], stil[:], bcP[:])
                    atcP = psg.tile([128, 1], f32, tag="g")
                    nc.tensor.transpose(atcP[:], atot[:], eye[0:1, 0:1])
                    atc = ckp.tile([128, 1], f32, tag="atc")
                    nc.any.tensor_copy(atc[:], atcP[:])
                    emcP = psg.tile([128, 1], f32, tag="g")
                    nc.tensor.transpose(emcP[:], emid[:], eye[0:1, 0:1])
                    emc = ckp.tile([128, 1], f32, tag="emc")
                    nc.any.tensor_copy(emc[:], emcP[:])
                    # scaled states (hkT*emid -> transpose; hv*emid)
                    hkTs = ck1.tile([128, 256], bf16, tag="hkTs")
                    nc.vector.tensor_scalar_mul(hkTs[:], state[:, 0:256],
                                                emc[:])
                    hv = ck1.tile([128, 256], bf16, tag="hv")
                    nc.vector.tensor_scalar_mul(hv[:], state[:, 256:512],
                                                emc[:])
                    hkn = ck1.tile([128, 256], bf16, tag="hkn")
                    for kt in range(2):
                        hknP = pst.tile([128, 128], bf16, tag="tp")
                        nc.tensor.transpose(
                            hknP[:], hkTs[:, kt * 128:(kt + 1) * 128], eye_bf)
                        nc.any.tensor_copy(
                            hkn[:, kt * 128:(kt + 1) * 128], hknP[:])
                    # k natural from kT transposes: [C, (h,kt) 128] x4
                    knat = ck1.tile([128, 512], bf16, tag="knat")
                    for h in range(HP):
                        for kt in range(2):
                            cc = 2 * h + kt
                            knP = pst.tile([128, 128], bf16, tag="tp")
                            nc.tensor.transpose(
                                knP[:], kT[:, cc * T + t0:cc * T + t0 + C],
                                eye_bf)
                            nc.any.tensor_copy(
                                knat[:, h * 256 + kt * 128:
                                     h * 256 + (kt + 1) * 128], knP[:])
                    # KQ grams + logits
                    lgP = pslg.tile([C, 128], f32, tag="lgP")
                    kqm = ck1.tile([C, 2 * C], bf16, tag="kqm")
                    for h in range(HP):
                        kqP = psgr.tile([C, C], f32, tag="gram")
                        for kt in range(2):
                            cc = 2 * h + kt
                            nc.tensor.matmul(
                                kqP[:], kT[:, cc * T + t0:cc * T + t0 + C],
                                qT[:, cc * T + t0:cc * T + t0 + C],
                                start=(kt == 0), stop=(kt == 1))
                        nc.vector.tensor_mul(
                            kqm[:, h * C:(h + 1) * C], kqP[:], mask_ut)
                        for kt in range(2):
                            cc = 2 * h + kt
                            nc.tensor.matmul(
                                lgP[:, h * M:(h + 1) * M],
                                qT[:, cc * T + t0:cc * T + t0 + C],
                                hkn[:, kt * 128 + h * M:
                                    kt * 128 + (h + 1) * M],
                                start=(kt == 0), stop=False)
                        nc.tensor.matmul(
                            lgP[:, h * M:(h + 1) * M],
                            kqm[:, h * C:(h + 1) * C],
                            stil[:, h * M:(h + 1) * M],
                            start=False, stop=True)
                    lg = ckp.tile([C, 128], f32, tag="lg")
                    nc.vector.tensor_mul(lg[:], lgP[:], ai[:])
                    # softmax over M per head; pt = p * ai
                    pt = ckp.tile([C, 128], bf16, tag="pt")
                    for h in range(HP):
                        hs = slice(h * M, (h + 1) * M)
                        nmax = ckp.tile([C, 1], f32, tag="nmax")
                        nc.vector.tensor_reduce(nmax[:], lg[:, hs], axis=AX.X,
                                                op=ALU.max, negate=True)
                        bias = ckp.tile([C, 1], f32, tag="bias")
                        nc.vector.tensor_scalar_mul(bias[:], nmax[:], SCALE)
                        e = ckp.tile([C, M], f32, tag="e")
                        esum = ckp.tile([C, 1], f32, tag="esum")
                        nc.scalar.activation(e[:], lg[:, hs], AF.Exp,
                                             bias=bias[:], scale=SCALE,
                                             accum_out=esum[:])
                        rcp = ckp.tile([C, 1], f32, tag="rcp")
                        nc.vector.reciprocal(rcp[:], esum[:])
                        p1 = ckp.tile([C, M], f32, tag="p1")
                        nc.vector.tensor_scalar_mul(p1[:], e[:], rcp[:])
                        nc.vector.tensor_mul(pt[:, hs], p1[:], ai[:, hs])
                    ptT = ck1.tile([128, C], bf16, tag="ptT")
                    ptTP = pst.tile([128, C], bf16, tag="tp")
                    nc.tensor.transpose(ptTP[:], pt[:], eye_bf)
                    nc.any.tensor_copy(ptT[:], ptTP[:])
                    stT = ck1.tile([128, C], bf16, tag="stT")
                    stTP = pst.tile([128, C], bf16, tag="tp")
                    nc.tensor.transpose(stTP[:], stil[:], eye_bf)
                    nc.any.tensor_copy(stT[:], stTP[:])
                    # o = pt @ hv + tril(PS) @ v
                    oP = psbig.tile([C, 512], f32, tag="big")
                    for h in range(HP):
                        psP = psgr.tile([C, C], f32, tag="gram")
                        nc.tensor.matmul(psP[:], stT[h * M:(h + 1) * M, :],
                                         ptT[h * M:(h + 1) * M, :])
                        psm = ck1.tile([C, C], bf16, tag="psm")
                        nc.vector.tensor_mul(psm[:], psP[:], mask_ut)
                        vs = slice(h * V, (h + 1) * V)
                        nc.tensor.matmul(
                            oP[:, vs], ptT[h * M:(h + 1) * M, :],
                            hv[h * M:(h + 1) * M, :], start=True, stop=False)
                        nc.tensor.matmul(
                            oP[:, vs], psm[:],
                            vn[:, i * KP + h * V:i * KP + (h + 1) * V],
                            start=False, stop=True)
                    # state update: U then scan step
                    uP = psbig.tile([128, 512], f32, tag="big")
                    for h in range(HP):
                        hp = slice(h * M, (h + 1) * M)
                        nc.tensor.matmul(uP[h * M:(h + 1) * M, 0:256],
                                         sa[:, hp],
                                         knat[:, h * 256:(h + 1) * 256])
                        nc.tensor.matmul(uP[h * M:(h + 1) * M, 256:512],
                                         sa[:, hp],
                                         vn[:, i * KP + h * V:
                                            i * KP + (h + 1) * V])
                    dec = ck1.tile([128, 512], f32, tag="dec")
                    nc.vector.tensor_scalar_mul(dec[:], state[:], atc[:])
                    state_n = stp.tile([128, 512], f32, tag="state")
                    nc.vector.tensor_add(state_n[:], dec[:], uP[:])
                    state = state_n
                    # RMSNorm from PSUM, write o_bf, transpose to oT
                    obf = ck1.tile([C, 512], bf16, tag="obf")
                    for h in range(HP):
                        vs = slice(h * V, (h + 1) * V)
                        sq = ck1.tile([C, V], bf16, tag="sq")
                        ssq = ckp.tile([C, 1], f32, tag="ssq")
                        nc.scalar.activation(sq[:], oP[:, vs], AF.Square,
                                             accum_out=ssq[:])
                        rms = ckp.tile([C, 1], f32, tag="rms")
                        nc.scalar.activation(rms[:], ssq[:], AF.Sqrt,
                                             bias=epsc[:], scale=1.0 / V)
                        rrms = ckp.tile([C, 1], f32, tag="rrms")
                        nc.vector.reciprocal(rrms[:], rms[:])
                        nc.vector.tensor_scalar_mul(obf[:, vs], oP[:, vs],
                                                    rrms[:])
                    for cc in range(4):
                        oTP = pst.tile([128, C], bf16, tag="tp")
                        nc.tensor.transpose(
                            oTP[:], obf[:, cc * 128:(cc + 1) * 128], eye_bf)
                        nc.any.tensor_copy(oT[:, cc * T + t0:cc * T + t0 + C],
                                           oTP[:])

            # ---- y = o @ wo ----
            with (
                tc.tile_pool(name="yp", bufs=3) as yp,
                tc.tile_pool(name="ypp", bufs=4, space="PSUM") as ypp,
            ):
                for tt in range(NCH):
                    t0 = tt * 128
                    ysb = yp.tile([128, D], f32, tag="ysb")
                    for half in range(2):
                        ps = ypp.tile([128, 512], f32, tag="yps")
                        for kt in range(4):
                            nc.tensor.matmul(
                                ps[:], oT[:, kt * T + t0:kt * T + t0 + 128],
                                wo[:, kt * D + half * 512:
                                   kt * D + (half + 1) * 512],
                                start=(kt == 0), stop=(kt == 3))
                        nc.any.tensor_copy(
                            ysb[:, half * 512:(half + 1) * 512], ps[:])
                    nc.sync.dma_start(y_d[t0:t0 + 128, :], ysb[:])

    nc.compile()
    return nc


def _get_program():
    if "nc" not in _cache:
        _cache["nc"] = _build_program()
    return _cache["nc"]


def kernel(x, Wq, Wk, Wv, Wf, g_norm_w, Wo):
    global last_exec_time_ns
    from concourse import bass_utils
    import ml_dtypes

    x = np.asarray(x, np.float32)
    Wq = np.asarray(Wq, np.float32)
    Wk = np.asarray(Wk, np.float32)
    Wv = np.asarray(Wv, np.float32)
    Wf = np.asarray(Wf, np.float32)
    gw = np.asarray(g_norm_w, np.float32)
    Wo = np.asarray(Wo, np.float32)

    consts = _build_consts()
    gw_full = np.tile(gw, HP)                       # [512]
    in_maps = []
    for core in range(8):
        b, hp = core // 2, core % 2
        xT = np.ascontiguousarray(x[b].T)           # [D, T]
        sk = slice(hp * KP, (hp + 1) * KP)
        sf = slice(hp * MP, (hp + 1) * MP)
        wo_s = (Wo[sk, :] * gw_full[:, None]).astype(ml_dtypes.bfloat16)
        in_maps.append({
            "xT": xT,
            "wq": np.ascontiguousarray(Wq[:, sk]),
            "wk": np.ascontiguousarray(Wk[:, sk]),
            "wv": np.ascontiguousarray(Wv[:, sk]),
            "wf": np.ascontiguousarray(Wf[:, sf]),
            "wo": wo_s,
            "cn": consts,
        })

    nc = _get_program()
    res = bass_utils.run_bass_kernel_spmd(nc, in_maps, core_ids=list(range(8)))
    if res.exec_time_ns is not None:
        last_exec_time_ns = res.exec_time_ns

    y = np.empty((B, T, D), np.float32)
    for b in range(B):
        y[b] = res.results[2 * b]["y"] + res.results[2 * b + 1]["y"]
    return y
